# revision 8
# baseline (speedup 1.0000x reference)
"""DOnePoleCell (one-pole IIR filter) Trainium2 Bass kernel.

Recurrence (per independent channel n, over time t):
    out[t] = b0*x[t] + s[t]
    s[t+1] = b1*x[t] + a1*out[t] = a1*s[t] + c*x[t],   c = b1 + a1*b0

Strategy:
  - Shard B=4096 channels across 8 cores (512 each), time kept whole.
  - On each core, time is cut into 129 chunks of 127 steps (+ a 1-step tail);
    each chunk is ONE fp32 [128,128]x[128,512] matmul on the PE. The moving
    tile's partition 0 carries the running state vector S, partitions 1..127
    carry the chunk's x rows. Column 0 of the stationary matrix emits the
    next state (into PSUM partition 0), columns 1..127 emit the 127 outputs.
    A single [1,512] PSUM->SBUF copy feeds the state into the next chunk's
    moving tile, so the whole scan is ~130 matmuls + 130 small copies.
    (Compute-engine APs must start at partition 0 -- hence carry-at-row-0.)
  - The serial carry chain is cut into P independent time segments; each
    non-first segment runs `w` warmup chunks starting from zero state (state
    influence decays as a1^127 per chunk, so for |a1| away from 1 a single
    warmup chunk reconstructs the state to fp32 exactness). For |a1| ~ 1 the
    plan degrades gracefully to fewer/one segment.
  - Nonzero initial state (never the case in the harness) is applied as a
    host-side rank-1 correction out[t] += a1^t * s0.
"""

import math

import numpy as np

T = 16384
B = 4096
NCORES = 8
NCB = B // NCORES  # 512 channels per core
L1 = 127  # steps per main chunk (127 x rows + 1 carry row)
NCHUNK = T // L1  # 129 main chunks
TAIL = T - NCHUNK * L1  # 1 leftover timestep

assert NCHUNK * L1 + TAIL == T and 0 < TAIL < L1

_PROGRAM_CACHE: dict = {}


def _chunk_rows(j: int) -> tuple[int, int]:
    """DRAM row range [r0, r1) of main chunk j."""
    return j * L1, (j + 1) * L1


def _segment_plan(a1c: float) -> tuple[int, int]:
    """Pick (n_segments P, warmup chunks w) from the runtime decay rate."""
    aa = abs(a1c)
    if aa >= 1.0 - 1e-12:
        return 1, 0  # no decay: strictly serial chain
    if aa < 1e-30:
        w = 1
    else:
        # want aa**(L1*w) <= 1e-7
        w = max(1, math.ceil(math.log(1e-7) / (L1 * math.log(aa))))
    for p in (4, 3, 2):
        if (p - 1) * w <= 32:
            return p, w
    return 1, 0


def _build_matrices(a1c: float, b0: float, b1: float):
    """G [128,128] for main chunks, GL [2,2] for the 1-step tail chunk.

    Moving rows: 0 = state S, 1+k = x[k].  Columns: 0 = next state,
    1+m = out[m].
    """
    a = float(a1c)
    c = b1 + a * b0
    g = np.zeros((128, 128), dtype=np.float64)
    k = np.arange(L1)
    # column 0: S' = a1^127 * S + sum_k c*a1^(126-k) x[k]
    g[0, 0] = a**L1
    g[1:, 0] = c * np.power(a, (L1 - 1 - k).astype(np.float64))
    # column 1+m: out[m] = a1^m * S + b0 x[m] + sum_{k<m} c a1^(m-1-k) x[k]
    g[0, 1:] = np.power(a, k.astype(np.float64))
    kk = k[:, None]
    mm = k[None, :]
    ee = mm - 1 - kk
    pw = np.where(ee >= 0, np.power(a, np.maximum(ee, 0).astype(np.float64)), 0.0)
    g[1:, 1:] = c * np.where(kk < mm, pw, 0.0) + b0 * np.eye(L1)
    # tail chunk: rows [S, x], cols [S', out]
    gl = np.array([[a, 1.0], [c, b0]], dtype=np.float64)
    return g.astype(np.float32), gl.astype(np.float32)


def _build_program(n_seg: int, n_warm: int):
    """Emit the Bass/Tile program. Structure depends only on (P, w)."""
    from concourse import mybir, tile
    from concourse.bacc import Bacc

    f32 = mybir.dt.float32

    nc = Bacc("TRN2", name="onepole")
    x_d = nc.dram_tensor("x", [T, NCB], f32, kind="ExternalInput")
    g_d = nc.dram_tensor("g", [128, 128], f32, kind="ExternalInput")
    gl_d = nc.dram_tensor("gl", [2, 2], f32, kind="ExternalInput")
    out_d = nc.dram_tensor("out", [T, NCB], f32, kind="ExternalOutput")
    fs_d = nc.dram_tensor("fs", [1, NCB], f32, kind="ExternalOutput")

    # segment boundaries over main-chunk indices [0, NCHUNK)
    bounds = [round(s * NCHUNK / n_seg) for s in range(n_seg + 1)]

    # per-segment step list: (chunk_idx, is_warmup)
    seg_steps = []
    for s in range(n_seg):
        steps = []
        if s > 0:
            for jw in range(bounds[s] - n_warm, bounds[s]):
                steps.append((jw, True))
        for j in range(bounds[s], bounds[s + 1]):
            steps.append((j, False))
        seg_steps.append(steps)

    with tile.TileContext(nc) as tc:
        with (
            tc.tile_pool(name="gpool", bufs=1) as gpool,
            tc.tile_pool(name="xpool", bufs=16) as xpool,
            tc.tile_pool(name="spool", bufs=10) as spool,
            tc.tile_pool(name="ppool", bufs=8, space="PSUM") as ppool,
        ):
            g_t = gpool.tile([128, 128], f32, name="gt")
            nc.sync.dma_start(g_t[:, :], g_d[:, :])
            gl_t = gpool.tile([2, 2], f32, name="glt")
            nc.sync.dma_start(gl_t[:, :], gl_d[:, :])

            # first moving tile of each segment: zero carry row + x rows
            cur_tiles = []
            for s in range(n_seg):
                j0, _ = seg_steps[s][0]
                r0, r1 = _chunk_rows(j0)
                t0 = xpool.tile([128, NCB], f32, tag="xt", name=f"xt_s{s}_first")
                nc.sync.dma_start(t0[1:128, :], x_d[r0:r1, :])
                nc.vector.memset(t0[0:1, :], 0.0)
                cur_tiles.append(t0)

            # interleaved emission, one step per segment per round
            max_steps = max(len(st) for st in seg_steps)
            for step_i in range(max_steps):
                for s in range(n_seg):
                    if step_i >= len(seg_steps[s]):
                        continue
                    j, warm = seg_steps[s][step_i]
                    xt = cur_tiles[s]
                    last_step = step_i == len(seg_steps[s]) - 1

                    # prefetch next step's x rows
                    nxt = None
                    if not last_step:
                        jn, _ = seg_steps[s][step_i + 1]
                        rn0, rn1 = _chunk_rows(jn)
                        nxt = xpool.tile([128, NCB], f32, tag="xt", name=f"xt_s{s}_{step_i + 1}")
                        nc.sync.dma_start(nxt[1:128, :], x_d[rn0:rn1, :])
                        cur_tiles[s] = nxt

                    ps = ppool.tile([128, NCB], f32, tag="ps", name=f"ps_s{s}_{step_i}")
                    nc.tensor.matmul(ps[:, :], g_t[:, :], xt[:, :], start=True, stop=True)

                    # carry into the next chunk's moving tile (row 0)
                    if not last_step:
                        nc.scalar.copy(nxt[0:1, :], ps[0:1, :])

                    # evict + store outputs (skip for warmup chunks)
                    if not warm:
                        r0, r1 = _chunk_rows(j)
                        st = spool.tile([128, NCB], f32, tag="st", name=f"st_s{s}_{step_i}")
                        nc.vector.tensor_copy(st[0:128, :], ps[0:128, :])
                        nc.sync.dma_start(out_d[r0:r1, :], st[1:128, :])

                    # tail: last main chunk feeds the 1-step tail chunk
                    if j == NCHUNK - 1:
                        tl = xpool.tile([128, NCB], f32, tag="xt", name="xt_tail")
                        nc.sync.dma_start(tl[1:2, :], x_d[T - TAIL : T, :])
                        nc.scalar.copy(tl[0:1, :], ps[0:1, :])
                        psf = ppool.tile([2, NCB], f32, tag="ps", name="ps_tail")
                        nc.tensor.matmul(
                            psf[0:2, :], gl_t[0:2, 0:2], tl[0:2, :], start=True, stop=True
                        )
                        stf = spool.tile([2, NCB], f32, name="st_tail")
                        nc.vector.tensor_copy(stf[0:2, :], psf[0:2, :])
                        nc.sync.dma_start(out_d[T - TAIL : T, :], stf[1:2, :])
                        nc.sync.dma_start(fs_d[0:1, :], stf[0:1, :])

    nc.compile()
    return nc


def _get_program(n_seg: int, n_warm: int):
    key = (n_seg, n_warm)
    if key not in _PROGRAM_CACHE:
        _PROGRAM_CACHE[key] = _build_program(n_seg, n_warm)
    return _PROGRAM_CACHE[key]


LAST_RESULT = None  # BassKernelResults of the most recent run (for test.py)
TRACE = False


def kernel(x, state, b0, b1, a1):
    global LAST_RESULT
    try:
        import bass_utils
    except ModuleNotFoundError:
        from concourse import bass_utils

    x = np.ascontiguousarray(np.asarray(x, dtype=np.float32))
    state = np.asarray(state, dtype=np.float32)
    b0f = float(np.asarray(b0).reshape(-1)[0])
    b1f = float(np.asarray(b1).reshape(-1)[0])
    a1f = float(np.asarray(a1).reshape(-1)[0])
    a1c = min(max(a1f, -1.0), 1.0)

    assert x.shape == (T, B), x.shape

    n_seg, n_warm = _segment_plan(a1c)
    g, gl = _build_matrices(a1c, b0f, b1f)
    nc = _get_program(n_seg, n_warm)

    in_maps = []
    for core in range(NCORES):
        sl = x[:, core * NCB : (core + 1) * NCB]
        in_maps.append({"x": np.ascontiguousarray(sl), "g": g, "gl": gl})

    res = bass_utils.run_bass_kernel_spmd(
        nc, in_maps, core_ids=list(range(NCORES)), trace=TRACE
    )
    LAST_RESULT = res

    out = np.concatenate([r["out"] for r in res.results], axis=1)
    fs = np.concatenate([r["fs"][0] for r in res.results], axis=0)

    if np.any(state):
        # host-side rank-1 correction for nonzero initial state
        pw = np.power(np.float64(a1c), np.arange(T, dtype=np.float64)).astype(np.float32)
        out = out + pw[:, None] * state[None, :]
        fs = fs + np.float32(float(a1c) ** T) * state

    return out.astype(np.float32), fs.astype(np.float32)


# revision 9
# speedup vs baseline: 7.4804x; 7.4804x over previous
"""DOnePoleCell (one-pole IIR filter) Trainium2 Bass kernel.

Recurrence (per independent channel n, over time t):
    out[t] = b0*x[t] + s[t]
    s[t+1] = b1*x[t] + a1*out[t] = a1*s[t] + c*x[t],   c = b1 + a1*b0

Strategy:
  - Shard B=4096 channels across 8 cores (512 each), time kept whole.
  - On each core, time is cut into 129 chunks of 127 steps (+ a 1-step tail);
    each chunk is ONE fp32 [128,128]x[128,512] matmul on the PE. The moving
    tile's partition 0 carries the running state vector S, partitions 1..127
    carry the chunk's x rows. Column 0 of the stationary matrix emits the
    next state (into PSUM partition 0), columns 1..127 emit the 127 outputs.
    A single [1,512] PSUM->SBUF copy feeds the state into the next chunk's
    moving tile, so the whole scan is ~130 matmuls + 130 small copies.
  - DMA efficiency: compute-engine APs must start at partition 0, and DMAs
    touching a partition range not starting at 0 run ~15x slower (measured).
    So both x and out use a host-padded [130*128, 512] chunk layout in DRAM
    where each chunk block is a full 128 partitions (row 0 = state slot).
    Chunks are batched 4 per DMA (1 MiB aligned transfers); input DMAs ride
    the sync (SP) HWDGE queue, output DMAs the gpsimd SWDGE queue.
  - The serial carry chain is cut into P independent time segments; each
    non-first segment runs `w` warmup chunks starting from zero state (state
    influence decays as a1^127 per chunk). For |a1| ~ 1 the plan degrades
    gracefully to fewer/one segment.
  - Nonzero initial state (never the case in the harness) is applied as a
    host-side rank-1 correction out[t] += a1^t * s0.
"""

import math

import numpy as np

T = 16384
B = 4096
NCORES = 8
NCB = B // NCORES  # 512 channels per core
L1 = 127  # steps per main chunk (127 x rows + 1 carry row)
NCHUNK = T // L1  # 129 main chunks
TAIL = T - NCHUNK * L1  # 1 leftover timestep
PADROWS = (NCHUNK + 1) * 128  # padded chunk layout rows (incl. tail block)
DMA_BATCH = 4  # chunks per DMA (1 MiB)

assert NCHUNK * L1 + TAIL == T and 0 < TAIL < L1

_PROGRAM_CACHE: dict = {}


def _segment_plan(a1c: float) -> tuple[int, int]:
    """Pick (n_segments P, warmup chunks w) from the runtime decay rate."""
    aa = abs(a1c)
    if aa >= 1.0 - 1e-12:
        return 1, 0  # no decay: strictly serial chain
    if aa < 1e-30:
        w = 1
    else:
        # want aa**(L1*w) <= 1e-7
        w = max(1, math.ceil(math.log(1e-7) / (L1 * math.log(aa))))
    for p in (4, 3, 2):
        if (p - 1) * w <= 32:
            return p, w
    return 1, 0


def _build_matrices(a1c: float, b0: float, b1: float):
    """G [128,128] for main chunks, GL [2,2] for the 1-step tail chunk.

    Moving rows: 0 = state S, 1+k = x[k].  Columns: 0 = next state,
    1+m = out[m].
    """
    a = float(a1c)
    c = b1 + a * b0
    g = np.zeros((128, 128), dtype=np.float64)
    k = np.arange(L1)
    # column 0: S' = a1^127 * S + sum_k c*a1^(126-k) x[k]
    g[0, 0] = a**L1
    g[1:, 0] = c * np.power(a, (L1 - 1 - k).astype(np.float64))
    # column 1+m: out[m] = a1^m * S + b0 x[m] + sum_{k<m} c a1^(m-1-k) x[k]
    g[0, 1:] = np.power(a, k.astype(np.float64))
    kk = k[:, None]
    mm = k[None, :]
    ee = mm - 1 - kk
    pw = np.where(ee >= 0, np.power(a, np.maximum(ee, 0).astype(np.float64)), 0.0)
    g[1:, 1:] = c * np.where(kk < mm, pw, 0.0) + b0 * np.eye(L1)
    # tail chunk: rows [S, x], cols [S', out]
    gl = np.array([[a, 1.0], [c, b0]], dtype=np.float64)
    return g.astype(np.float32), gl.astype(np.float32)


def _build_program(n_seg: int, n_warm: int):
    """Emit the Bass/Tile program. Structure depends only on (P, w)."""
    from concourse import mybir, tile
    from concourse.bacc import Bacc

    f32 = mybir.dt.float32

    nc = Bacc("TRN2", name="onepole")
    x_d = nc.dram_tensor("xp", [PADROWS, NCB], f32, kind="ExternalInput")
    g_d = nc.dram_tensor("g", [128, 128], f32, kind="ExternalInput")
    gl_d = nc.dram_tensor("gl", [2, 2], f32, kind="ExternalInput")
    out_d = nc.dram_tensor("outp", [PADROWS, NCB], f32, kind="ExternalOutput")

    # segment boundaries over main-chunk indices [0, NCHUNK)
    bounds = [round(s * NCHUNK / n_seg) for s in range(n_seg + 1)]

    # per-segment step list -> batches of up to DMA_BATCH chunks
    seg_batches = []
    for s in range(n_seg):
        steps = []
        if s > 0:
            for jw in range(bounds[s] - n_warm, bounds[s]):
                steps.append((jw, True))
        for j in range(bounds[s], bounds[s + 1]):
            steps.append((j, False))
        seg_batches.append(
            [steps[i : i + DMA_BATCH] for i in range(0, len(steps), DMA_BATCH)]
        )

    def in_ap(batch):
        """Aligned DRAM source AP covering the batch's chunk blocks."""
        j0 = batch[0][0]
        bs = len(batch)
        # chunk blocks are contiguous in x_pad (consecutive chunk indices)
        sl = x_d[j0 * 128 : (j0 + bs) * 128, :]
        return sl.rearrange("(c k) n -> k c n", c=bs)

    def out_ap(j0, bs):
        sl = out_d[j0 * 128 : (j0 + bs) * 128, :]
        return sl.rearrange("(c k) n -> k c n", c=bs)

    with tile.TileContext(nc) as tc:
        with (
            tc.tile_pool(name="gpool", bufs=1) as gpool,
            tc.tile_pool(name="xpool", bufs=6) as xpool,
            tc.tile_pool(name="spool", bufs=4) as spool,
            tc.tile_pool(name="ppool", bufs=8, space="PSUM") as ppool,
        ):
            g_t = gpool.tile([128, 128], f32, name="gt")
            nc.sync.dma_start(g_t[:, :], g_d[:, :])
            gl_t = gpool.tile([2, 2], f32, name="glt")
            nc.sync.dma_start(gl_t[:, :], gl_d[:, :])

            # first batch tile of each segment: load + zero the carry slot
            cur_tiles = []
            for s in range(n_seg):
                b0 = seg_batches[s][0]
                t0 = xpool.tile([128, DMA_BATCH, NCB], f32, tag="xt", name=f"xt_s{s}_b0")
                nc.sync.dma_start(t0[:, 0 : len(b0), :], in_ap(b0))
                nc.vector.memset(t0[0:1, 0, :], 0.0)
                cur_tiles.append(t0)

            max_batches = max(len(sb) for sb in seg_batches)
            for bi in range(max_batches):
                for s in range(n_seg):
                    if bi >= len(seg_batches[s]):
                        continue
                    batch = seg_batches[s][bi]
                    xt = cur_tiles[s]
                    last_batch = bi == len(seg_batches[s]) - 1

                    # prefetch next batch
                    nxt = None
                    if not last_batch:
                        nb = seg_batches[s][bi + 1]
                        nxt = xpool.tile(
                            [128, DMA_BATCH, NCB], f32, tag="xt", name=f"xt_s{s}_b{bi + 1}"
                        )
                        nc.sync.dma_start(nxt[:, 0 : len(nb), :], in_ap(nb))
                        cur_tiles[s] = nxt

                    st = spool.tile(
                        [128, DMA_BATCH, NCB], f32, tag="st", name=f"st_s{s}_b{bi}"
                    )
                    n_out = 0  # chunks of this batch that produce output
                    for ci, (j, warm) in enumerate(batch):
                        ps = ppool.tile([128, NCB], f32, tag="ps", name=f"ps_s{s}_b{bi}_{ci}")
                        nc.tensor.matmul(
                            ps[:, :], g_t[:, :], xt[:, ci, :], start=True, stop=True
                        )

                        # carry into the next chunk's moving slice (row 0)
                        last_chunk = last_batch and ci == len(batch) - 1
                        if not last_chunk:
                            if ci + 1 < len(batch):
                                nc.scalar.copy(xt[0:1, ci + 1, :], ps[0:1, :])
                            else:
                                nc.scalar.copy(nxt[0:1, 0, :], ps[0:1, :])

                        # evict everything (incl. the carry row -- stripped on host)
                        if not warm:
                            nc.vector.tensor_copy(st[:, ci, :], ps[:, :])
                            n_out += 1

                        # tail: last main chunk feeds the 1-step tail chunk
                        if j == NCHUNK - 1:
                            tl = xpool.tile([2, NCB], f32, name="xt_tail")
                            nc.sync.dma_start(
                                tl[0:2, :], x_d[NCHUNK * 128 : NCHUNK * 128 + 2, :]
                            )
                            nc.scalar.copy(tl[0:1, :], ps[0:1, :])
                            psf = ppool.tile([2, NCB], f32, tag="ps", name="ps_tail")
                            nc.tensor.matmul(
                                psf[0:2, :], gl_t[0:2, 0:2], tl[0:2, :], start=True, stop=True
                            )
                            stf = spool.tile([2, NCB], f32, name="st_tail")
                            nc.vector.tensor_copy(stf[0:2, :], psf[0:2, :])
                            nc.gpsimd.dma_start(
                                out_d[NCHUNK * 128 : NCHUNK * 128 + 2, :], stf[0:2, :]
                            )

                    if n_out:
                        # warmup chunks only occur as a prefix of a segment
                        j_first = batch[len(batch) - n_out][0]
                        nc.gpsimd.dma_start(
                            out_ap(j_first, n_out),
                            st[:, len(batch) - n_out : len(batch), :],
                        )

    nc.compile()
    return nc


def _get_program(n_seg: int, n_warm: int):
    key = (n_seg, n_warm)
    if key not in _PROGRAM_CACHE:
        _PROGRAM_CACHE[key] = _build_program(n_seg, n_warm)
    return _PROGRAM_CACHE[key]


LAST_RESULT = None  # BassKernelResults of the most recent run (for test.py)
TRACE = False


def kernel(x, state, b0, b1, a1):
    global LAST_RESULT
    try:
        import bass_utils
    except ModuleNotFoundError:
        from concourse import bass_utils

    x = np.asarray(x, dtype=np.float32)
    state = np.asarray(state, dtype=np.float32)
    b0f = float(np.asarray(b0).reshape(-1)[0])
    b1f = float(np.asarray(b1).reshape(-1)[0])
    a1f = float(np.asarray(a1).reshape(-1)[0])
    a1c = min(max(a1f, -1.0), 1.0)

    assert x.shape == (T, B), x.shape

    n_seg, n_warm = _segment_plan(a1c)
    g, gl = _build_matrices(a1c, b0f, b1f)
    nc = _get_program(n_seg, n_warm)

    in_maps = []
    for core in range(NCORES):
        sl = x[:, core * NCB : (core + 1) * NCB]
        xp = np.zeros((PADROWS, NCB), dtype=np.float32)
        # chunk c block rows c*128: row 0 = state slot, rows 1..127 = x rows
        xp[: NCHUNK * 128].reshape(NCHUNK, 128, NCB)[:, 1:, :] = sl[: NCHUNK * L1].reshape(
            NCHUNK, L1, NCB
        )
        xp[NCHUNK * 128 + 1] = sl[T - 1]
        in_maps.append({"xp": xp, "g": g, "gl": gl})

    res = bass_utils.run_bass_kernel_spmd(
        nc, in_maps, core_ids=list(range(NCORES)), trace=TRACE
    )
    LAST_RESULT = res

    outs = []
    fss = []
    for r in res.results:
        op = r["outp"]
        main = op[: NCHUNK * 128].reshape(NCHUNK, 128, NCB)[:, 1:, :].reshape(
            NCHUNK * L1, NCB
        )
        last_row = op[NCHUNK * 128 + 1 : NCHUNK * 128 + 2]
        outs.append(np.concatenate([main, last_row], axis=0))
        fss.append(op[NCHUNK * 128])
    out = np.concatenate(outs, axis=1)
    fs = np.concatenate(fss, axis=0)

    if np.any(state):
        # host-side rank-1 correction for nonzero initial state
        pw = np.power(np.float64(a1c), np.arange(T, dtype=np.float64)).astype(np.float32)
        out = out + pw[:, None] * state[None, :]
        fs = fs + np.float32(float(a1c) ** T) * state

    return np.ascontiguousarray(out.astype(np.float32)), fs.astype(np.float32)


# revision 14
# speedup vs baseline: 11.9583x; 1.5986x over previous
"""DOnePoleCell (one-pole IIR filter) Trainium2 Bass kernel.

Recurrence (per independent channel n, over time t):
    out[t] = b0*x[t] + s[t]
    s[t+1] = b1*x[t] + a1*out[t] = a1*s[t] + c*x[t],   c = b1 + a1*b0

Strategy:
  - Shard B=4096 channels across 8 cores (512 each), time kept whole.
  - On each core, time is cut into 129 chunks of 127 steps (+ a 1-step tail);
    each chunk is ONE fp32 [128,128]x[128,512] matmul on the PE. The moving
    tile's partition 0 carries the running state vector S, partitions 1..127
    carry the chunk's x rows. Column 0 of the stationary matrix emits the
    next state (into PSUM partition 0), columns 1..127 emit the 127 outputs.
    A single [1,512] PSUM->SBUF copy feeds the state into the next chunk's
    moving tile, so the whole scan is ~130 matmuls + 130 small copies.
  - DMA efficiency: compute-engine APs must start at partition 0, and DMAs
    touching a partition range not starting at 0 run ~15x slower (measured).
    So both x and out use a host-padded [130*128, 512] chunk layout in DRAM
    where each chunk block is a full 128 partitions (row 0 = state slot).
    Chunks are batched 4 per DMA (1 MiB aligned transfers); input DMAs ride
    the sync (SP) HWDGE queue, output DMAs the gpsimd SWDGE queue.
  - The serial carry chain is cut into P independent time segments; each
    non-first segment runs `w` warmup chunks starting from zero state (state
    influence decays as a1^127 per chunk). For |a1| ~ 1 the plan degrades
    gracefully to fewer/one segment.
  - Nonzero initial state (never the case in the harness) is applied as a
    host-side rank-1 correction out[t] += a1^t * s0.
"""

import math

import numpy as np

T = 16384
B = 4096
NCORES = 8
NCB = B // NCORES  # 512 channels per core
L1 = 127  # steps per main chunk (127 x rows + 1 carry row)
NCHUNK = T // L1  # 129 main chunks
TAIL = T - NCHUNK * L1  # 1 leftover timestep
PADROWS = (NCHUNK + 1) * 128  # padded chunk layout rows (incl. tail block)
DMA_BATCH = 4  # chunks per DMA (1 MiB)

assert NCHUNK * L1 + TAIL == T and 0 < TAIL < L1

_PROGRAM_CACHE: dict = {}


def _segment_plan(a1c: float) -> tuple[int, int]:
    """Pick (n_segments P, warmup chunks w) from the runtime decay rate."""
    aa = abs(a1c)
    if aa >= 1.0 - 1e-12:
        return 1, 0  # no decay: strictly serial chain
    if aa < 1e-30:
        w = 1
    else:
        # want aa**(L1*w) <= 1e-7
        w = max(1, math.ceil(math.log(1e-7) / (L1 * math.log(aa))))
    for p in (4, 3, 2):
        if (p - 1) * w <= 32:
            return p, w
    return 1, 0


def _build_matrices(a1c: float, b0: float, b1: float):
    """G [128,128] for main chunks, GL [2,2] for the 1-step tail chunk.

    Moving rows: 0 = state S, 1+k = x[k].  Columns: 0 = next state,
    1+m = out[m].
    """
    a = float(a1c)
    c = b1 + a * b0
    g = np.zeros((128, 128), dtype=np.float64)
    k = np.arange(L1)
    # column 0: S' = a1^127 * S + sum_k c*a1^(126-k) x[k]
    g[0, 0] = a**L1
    g[1:, 0] = c * np.power(a, (L1 - 1 - k).astype(np.float64))
    # column 1+m: out[m] = a1^m * S + b0 x[m] + sum_{k<m} c a1^(m-1-k) x[k]
    g[0, 1:] = np.power(a, k.astype(np.float64))
    kk = k[:, None]
    mm = k[None, :]
    ee = mm - 1 - kk
    pw = np.where(ee >= 0, np.power(a, np.maximum(ee, 0).astype(np.float64)), 0.0)
    g[1:, 1:] = c * np.where(kk < mm, pw, 0.0) + b0 * np.eye(L1)
    # tail chunk: rows [S, x], cols [S', out]
    gl = np.array([[a, 1.0], [c, b0]], dtype=np.float64)
    return g.astype(np.float32), gl.astype(np.float32)


def _build_program(n_seg: int, n_warm: int):
    """Emit the Bass/Tile program. Structure depends only on (P, w)."""
    from concourse import mybir, tile
    from concourse.bacc import Bacc

    f32 = mybir.dt.float32
    f32r = mybir.dt.float32r  # full-rate single-pass PE matmul (reduced precision)

    nc = Bacc("TRN2", name="onepole")
    x_d = nc.dram_tensor("xp", [PADROWS, NCB], f32r, kind="ExternalInput")
    g_d = nc.dram_tensor("g", [128, 128], f32r, kind="ExternalInput")
    gl_d = nc.dram_tensor("gl", [2, 2], f32r, kind="ExternalInput")
    out_d = nc.dram_tensor("outp", [PADROWS, NCB], f32, kind="ExternalOutput")

    # segment boundaries over main-chunk indices [0, NCHUNK)
    bounds = [round(s * NCHUNK / n_seg) for s in range(n_seg + 1)]

    # per-segment step list -> batches of up to DMA_BATCH chunks
    seg_batches = []
    for s in range(n_seg):
        steps = []
        if s > 0:
            for jw in range(bounds[s] - n_warm, bounds[s]):
                steps.append((jw, True))
        for j in range(bounds[s], bounds[s + 1]):
            steps.append((j, False))
        seg_batches.append(
            [steps[i : i + DMA_BATCH] for i in range(0, len(steps), DMA_BATCH)]
        )

    def in_ap(batch):
        """Aligned DRAM source AP covering the batch's chunk blocks."""
        j0 = batch[0][0]
        bs = len(batch)
        # chunk blocks are contiguous in x_pad (consecutive chunk indices)
        sl = x_d[j0 * 128 : (j0 + bs) * 128, :]
        return sl.rearrange("(c k) n -> k c n", c=bs)

    def out_ap(j0, bs):
        sl = out_d[j0 * 128 : (j0 + bs) * 128, :]
        return sl.rearrange("(c k) n -> k c n", c=bs)

    with tile.TileContext(nc) as tc:
        with (
            tc.tile_pool(name="gpool", bufs=1) as gpool,
            tc.tile_pool(name="xpool", bufs=6) as xpool,
            tc.tile_pool(name="spool", bufs=4) as spool,
            tc.tile_pool(name="ppool", bufs=8, space="PSUM") as ppool,
        ):
            g_t = gpool.tile([128, 128], f32r, name="gt")
            nc.sync.dma_start(g_t[:, :], g_d[:, :])
            gl_t = gpool.tile([2, 2], f32r, name="glt")
            nc.sync.dma_start(gl_t[:, :], gl_d[:, :])

            # first batch tile of each segment: load + zero the carry slot
            cur_tiles = []
            for s in range(n_seg):
                b0 = seg_batches[s][0]
                t0 = xpool.tile([128, DMA_BATCH, NCB], f32r, tag="xt", name=f"xt_s{s}_b0")
                nc.sync.dma_start(t0[:, 0 : len(b0), :], in_ap(b0))
                # no memset needed: x_pad's carry slots are zero-filled on host,
                # so the DMA itself delivers S=0 for each segment's first chunk
                cur_tiles.append(t0)

            max_batches = max(len(sb) for sb in seg_batches)
            for bi in range(max_batches):
                for s in range(n_seg):
                    if bi >= len(seg_batches[s]):
                        continue
                    batch = seg_batches[s][bi]
                    xt = cur_tiles[s]
                    last_batch = bi == len(seg_batches[s]) - 1

                    # prefetch next batch
                    nxt = None
                    if not last_batch:
                        nb = seg_batches[s][bi + 1]
                        nxt = xpool.tile(
                            [128, DMA_BATCH, NCB], f32r, tag="xt", name=f"xt_s{s}_b{bi + 1}"
                        )
                        nc.sync.dma_start(nxt[:, 0 : len(nb), :], in_ap(nb))
                        cur_tiles[s] = nxt

                    st = spool.tile(
                        [128, DMA_BATCH, NCB], f32, tag="st", name=f"st_s{s}_b{bi}"
                    )
                    n_out = 0  # chunks of this batch that produce output
                    for ci, (j, warm) in enumerate(batch):
                        ps = ppool.tile([128, NCB], f32, tag="ps", name=f"ps_s{s}_b{bi}_{ci}")
                        nc.tensor.matmul(
                            ps[:, :],
                            g_t[:, :],
                            xt[:, ci, :],
                            start=True,
                            stop=True,
                        )

                        # carry into the next chunk's moving slice (row 0)
                        last_chunk = last_batch and ci == len(batch) - 1
                        if not last_chunk:
                            if ci + 1 < len(batch):
                                nc.scalar.copy(xt[0:1, ci + 1, :], ps[0:1, :])
                            else:
                                nc.scalar.copy(nxt[0:1, 0, :], ps[0:1, :])

                        # evict everything (incl. the carry row -- stripped on host)
                        if not warm:
                            nc.vector.tensor_copy(st[:, ci, :], ps[:, :])
                            n_out += 1

                        # tail: last main chunk feeds the 1-step tail chunk
                        if j == NCHUNK - 1:
                            tl = xpool.tile([2, NCB], f32r, name="xt_tail")
                            nc.sync.dma_start(
                                tl[0:2, :], x_d[NCHUNK * 128 : NCHUNK * 128 + 2, :]
                            )
                            nc.scalar.copy(tl[0:1, :], ps[0:1, :])
                            psf = ppool.tile([2, NCB], f32, tag="ps", name="ps_tail")
                            nc.tensor.matmul(
                                psf[0:2, :],
                                gl_t[0:2, 0:2],
                                tl[0:2, :],
                                start=True,
                                stop=True,
                            )
                            stf = spool.tile([2, NCB], f32, name="st_tail")
                            nc.vector.tensor_copy(stf[0:2, :], psf[0:2, :])
                            nc.gpsimd.dma_start(
                                out_d[NCHUNK * 128 : NCHUNK * 128 + 2, :], stf[0:2, :]
                            )

                    if n_out:
                        # warmup chunks only occur as a prefix of a segment
                        j_first = batch[len(batch) - n_out][0]
                        nc.gpsimd.dma_start(
                            out_ap(j_first, n_out),
                            st[:, len(batch) - n_out : len(batch), :],
                        )

    nc.compile()
    return nc


def _get_program(n_seg: int, n_warm: int):
    key = (n_seg, n_warm)
    if key not in _PROGRAM_CACHE:
        _PROGRAM_CACHE[key] = _build_program(n_seg, n_warm)
    return _PROGRAM_CACHE[key]


LAST_RESULT = None  # BassKernelResults of the most recent run (for test.py)
TRACE = False


def kernel(x, state, b0, b1, a1):
    global LAST_RESULT
    try:
        import bass_utils
    except ModuleNotFoundError:
        from concourse import bass_utils

    x = np.asarray(x, dtype=np.float32)
    state = np.asarray(state, dtype=np.float32)
    b0f = float(np.asarray(b0).reshape(-1)[0])
    b1f = float(np.asarray(b1).reshape(-1)[0])
    a1f = float(np.asarray(a1).reshape(-1)[0])
    a1c = min(max(a1f, -1.0), 1.0)

    assert x.shape == (T, B), x.shape

    n_seg, n_warm = _segment_plan(a1c)
    g, gl = _build_matrices(a1c, b0f, b1f)
    nc = _get_program(n_seg, n_warm)

    in_maps = []
    for core in range(NCORES):
        sl = x[:, core * NCB : (core + 1) * NCB]
        xp = np.zeros((PADROWS, NCB), dtype=np.float32)
        # chunk c block rows c*128: row 0 = state slot, rows 1..127 = x rows
        xp[: NCHUNK * 128].reshape(NCHUNK, 128, NCB)[:, 1:, :] = sl[: NCHUNK * L1].reshape(
            NCHUNK, L1, NCB
        )
        xp[NCHUNK * 128 + 1] = sl[T - 1]
        in_maps.append({"xp": xp, "g": g, "gl": gl})

    res = bass_utils.run_bass_kernel_spmd(
        nc, in_maps, core_ids=list(range(NCORES)), trace=TRACE
    )
    LAST_RESULT = res

    outs = []
    fss = []
    for r in res.results:
        op = r["outp"]
        main = op[: NCHUNK * 128].reshape(NCHUNK, 128, NCB)[:, 1:, :].reshape(
            NCHUNK * L1, NCB
        )
        last_row = op[NCHUNK * 128 + 1 : NCHUNK * 128 + 2]
        outs.append(np.concatenate([main, last_row], axis=0))
        fss.append(op[NCHUNK * 128])
    out = np.concatenate(outs, axis=1)
    fs = np.concatenate(fss, axis=0)

    if np.any(state):
        # host-side rank-1 correction for nonzero initial state
        pw = np.power(np.float64(a1c), np.arange(T, dtype=np.float64)).astype(np.float32)
        out = out + pw[:, None] * state[None, :]
        fs = fs + np.float32(float(a1c) ** T) * state

    return np.ascontiguousarray(out.astype(np.float32)), fs.astype(np.float32)


# revision 15
# speedup vs baseline: 13.7422x; 1.1492x over previous
"""DOnePoleCell (one-pole IIR filter) Trainium2 Bass kernel.

Recurrence (per independent channel n, over time t):
    out[t] = b0*x[t] + s[t]
    s[t+1] = b1*x[t] + a1*out[t] = a1*s[t] + c*x[t],   c = b1 + a1*b0

Strategy:
  - Shard B=4096 channels across 8 cores (512 each), time kept whole.
  - On each core, time is cut into 129 chunks of 127 steps (+ a 1-step tail);
    each chunk is ONE fp32 [128,128]x[128,512] matmul on the PE. The moving
    tile's partition 0 carries the running state vector S, partitions 1..127
    carry the chunk's x rows. Column 0 of the stationary matrix emits the
    next state (into PSUM partition 0), columns 1..127 emit the 127 outputs.
    A single [1,512] PSUM->SBUF copy feeds the state into the next chunk's
    moving tile, so the whole scan is ~130 matmuls + 130 small copies.
  - DMA efficiency: compute-engine APs must start at partition 0, and DMAs
    touching a partition range not starting at 0 run ~15x slower (measured).
    So both x and out use a host-padded [130*128, 512] chunk layout in DRAM
    where each chunk block is a full 128 partitions (row 0 = state slot).
    Chunks are batched 4 per DMA (1 MiB aligned transfers); input DMAs ride
    the sync (SP) HWDGE queue, output DMAs the gpsimd SWDGE queue.
  - The serial carry chain is cut into P independent time segments; each
    non-first segment runs `w` warmup chunks starting from zero state (state
    influence decays as a1^127 per chunk). For |a1| ~ 1 the plan degrades
    gracefully to fewer/one segment.
  - Nonzero initial state (never the case in the harness) is applied as a
    host-side rank-1 correction out[t] += a1^t * s0.
"""

import math

import numpy as np

T = 16384
B = 4096
NCORES = 8
NCB = B // NCORES  # 512 channels per core
L1 = 127  # steps per main chunk (127 x rows + 1 carry row)
NCHUNK = T // L1  # 129 main chunks
TAIL = T - NCHUNK * L1  # 1 leftover timestep
PADROWS = (NCHUNK + 1) * 128  # padded chunk layout rows (incl. tail block)
DMA_BATCH = 4  # chunks per DMA (1 MiB)

assert NCHUNK * L1 + TAIL == T and 0 < TAIL < L1

_PROGRAM_CACHE: dict = {}


def _plan_batches(n_seg: int, n_warm: int):
    """Per-segment batches of (chunk_idx, is_warmup), plus each batch's block
    index in the batch-contiguous x layout. Shared by device program + host
    packing so the layouts always agree."""
    bounds = [round(s * NCHUNK / n_seg) for s in range(n_seg + 1)]
    seg_batches = []
    blk = 0
    for s in range(n_seg):
        steps = []
        if s > 0:
            for jw in range(bounds[s] - n_warm, bounds[s]):
                steps.append((jw, True))
        for j in range(bounds[s], bounds[s + 1]):
            steps.append((j, False))
        batches = []
        for i in range(0, len(steps), DMA_BATCH):
            batches.append((blk, steps[i : i + DMA_BATCH]))
            blk += 1
        seg_batches.append(batches)
    return seg_batches, blk  # blk = number of main blocks (tail block follows)


def _segment_plan(a1c: float) -> tuple[int, int]:
    """Pick (n_segments P, warmup chunks w) from the runtime decay rate."""
    aa = abs(a1c)
    if aa >= 1.0 - 1e-12:
        return 1, 0  # no decay: strictly serial chain
    if aa < 1e-30:
        w = 1
    else:
        # want aa**(L1*w) <= 1e-7
        w = max(1, math.ceil(math.log(1e-7) / (L1 * math.log(aa))))
    for p in (4, 3, 2):
        if (p - 1) * w <= 32:
            return p, w
    return 1, 0


def _build_matrices(a1c: float, b0: float, b1: float):
    """G [128,128] for main chunks, GL [2,2] for the 1-step tail chunk.

    Moving rows: 0 = state S, 1+k = x[k].  Columns: 0 = next state,
    1+m = out[m].
    """
    a = float(a1c)
    c = b1 + a * b0
    g = np.zeros((128, 128), dtype=np.float64)
    k = np.arange(L1)
    # column 0: S' = a1^127 * S + sum_k c*a1^(126-k) x[k]
    g[0, 0] = a**L1
    g[1:, 0] = c * np.power(a, (L1 - 1 - k).astype(np.float64))
    # column 1+m: out[m] = a1^m * S + b0 x[m] + sum_{k<m} c a1^(m-1-k) x[k]
    g[0, 1:] = np.power(a, k.astype(np.float64))
    kk = k[:, None]
    mm = k[None, :]
    ee = mm - 1 - kk
    pw = np.where(ee >= 0, np.power(a, np.maximum(ee, 0).astype(np.float64)), 0.0)
    g[1:, 1:] = c * np.where(kk < mm, pw, 0.0) + b0 * np.eye(L1)
    # tail chunk: rows [S, x], cols [S', out]
    gl = np.array([[a, 1.0], [c, b0]], dtype=np.float64)
    return g.astype(np.float32), gl.astype(np.float32)


def _build_program(n_seg: int, n_warm: int):
    """Emit the Bass/Tile program. Structure depends only on (P, w)."""
    from concourse import mybir, tile
    from concourse.bacc import Bacc

    f32 = mybir.dt.float32
    f32r = mybir.dt.float32r  # full-rate single-pass PE matmul (reduced precision)

    nc = Bacc("TRN2", name="onepole")
    seg_batches, n_blocks = _plan_batches(n_seg, n_warm)
    # batch-contiguous x: block b holds its batch's [128, bs*NCB] tile image
    # (so every in-DMA is a fully contiguous, 128-partition-aligned read);
    # one extra block at the end holds the tail chunk's 2 rows.
    x_d = nc.dram_tensor("xp", [(n_blocks + 1) * 128, DMA_BATCH * NCB], f32r, kind="ExternalInput")
    g_d = nc.dram_tensor("g", [128, 128], f32r, kind="ExternalInput")
    gl_d = nc.dram_tensor("gl", [2, 2], f32r, kind="ExternalInput")
    out_d = nc.dram_tensor("outp", [PADROWS, NCB], f32, kind="ExternalOutput")

    def in_ap(blk, bs):
        """Contiguous DRAM source AP for batch block blk (bs chunks)."""
        sl = x_d[blk * 128 : (blk + 1) * 128, 0 : bs * NCB]
        return sl.rearrange("k (c n) -> k c n", c=bs)

    def out_ap(j0, bs):
        sl = out_d[j0 * 128 : (j0 + bs) * 128, :]
        return sl.rearrange("(c k) n -> k c n", c=bs)

    with tile.TileContext(nc) as tc:
        with (
            tc.tile_pool(name="gpool", bufs=1) as gpool,
            tc.tile_pool(name="xpool", bufs=10) as xpool,
            tc.tile_pool(name="spool", bufs=5) as spool,
            tc.tile_pool(name="ppool", bufs=8, space="PSUM") as ppool,
        ):
            g_t = gpool.tile([128, 128], f32r, name="gt")
            nc.sync.dma_start(g_t[:, :], g_d[:, :])
            gl_t = gpool.tile([2, 2], f32r, name="glt")
            nc.sync.dma_start(gl_t[:, :], gl_d[:, :])

            # first batch tile of each segment: load + zero the carry slot
            cur_tiles = []
            for s in range(n_seg):
                blk0, b0 = seg_batches[s][0]
                t0 = xpool.tile([128, DMA_BATCH, NCB], f32r, tag="xt", name=f"xt_s{s}_b0")
                nc.sync.dma_start(t0[:, 0 : len(b0), :], in_ap(blk0, len(b0)))
                # no memset needed: x_pad's carry slots are zero-filled on host,
                # so the DMA itself delivers S=0 for each segment's first chunk
                cur_tiles.append(t0)

            max_batches = max(len(sb) for sb in seg_batches)
            for bi in range(max_batches):
                for s in range(n_seg):
                    if bi >= len(seg_batches[s]):
                        continue
                    _, batch = seg_batches[s][bi]
                    xt = cur_tiles[s]
                    last_batch = bi == len(seg_batches[s]) - 1

                    # prefetch next batch
                    nxt = None
                    if not last_batch:
                        nblk, nb = seg_batches[s][bi + 1]
                        nxt = xpool.tile(
                            [128, DMA_BATCH, NCB], f32r, tag="xt", name=f"xt_s{s}_b{bi + 1}"
                        )
                        nc.sync.dma_start(nxt[:, 0 : len(nb), :], in_ap(nblk, len(nb)))
                        cur_tiles[s] = nxt

                    st = spool.tile(
                        [128, DMA_BATCH, NCB], f32, tag="st", name=f"st_s{s}_b{bi}"
                    )
                    n_out = 0  # chunks of this batch that produce output
                    for ci, (j, warm) in enumerate(batch):
                        ps = ppool.tile([128, NCB], f32, tag="ps", name=f"ps_s{s}_b{bi}_{ci}")
                        nc.tensor.matmul(
                            ps[:, :],
                            g_t[:, :],
                            xt[:, ci, :],
                            start=True,
                            stop=True,
                        )

                        # carry into the next chunk's moving slice (row 0)
                        last_chunk = last_batch and ci == len(batch) - 1
                        if not last_chunk:
                            if ci + 1 < len(batch):
                                nc.scalar.copy(xt[0:1, ci + 1, :], ps[0:1, :])
                            else:
                                nc.scalar.copy(nxt[0:1, 0, :], ps[0:1, :])

                        # evict everything (incl. the carry row -- stripped on host)
                        if not warm:
                            nc.vector.tensor_copy(st[:, ci, :], ps[:, :])
                            n_out += 1

                        # tail: last main chunk feeds the 1-step tail chunk
                        if j == NCHUNK - 1:
                            tl = xpool.tile([2, NCB], f32r, name="xt_tail")
                            nc.sync.dma_start(
                                tl[0:2, :], x_d[n_blocks * 128 : n_blocks * 128 + 2, 0:NCB]
                            )
                            nc.scalar.copy(tl[0:1, :], ps[0:1, :])
                            psf = ppool.tile([2, NCB], f32, tag="ps", name="ps_tail")
                            nc.tensor.matmul(
                                psf[0:2, :],
                                gl_t[0:2, 0:2],
                                tl[0:2, :],
                                start=True,
                                stop=True,
                            )
                            stf = spool.tile([2, NCB], f32, name="st_tail")
                            nc.vector.tensor_copy(stf[0:2, :], psf[0:2, :])
                            nc.gpsimd.dma_start(
                                out_d[NCHUNK * 128 : NCHUNK * 128 + 2, :], stf[0:2, :]
                            )

                    if n_out:
                        # warmup chunks only occur as a prefix of a segment
                        j_first = batch[len(batch) - n_out][0]
                        nc.gpsimd.dma_start(
                            out_ap(j_first, n_out),
                            st[:, len(batch) - n_out : len(batch), :],
                        )

    nc.compile()
    return nc


def _get_program(n_seg: int, n_warm: int):
    key = (n_seg, n_warm)
    if key not in _PROGRAM_CACHE:
        _PROGRAM_CACHE[key] = _build_program(n_seg, n_warm)
    return _PROGRAM_CACHE[key]


LAST_RESULT = None  # BassKernelResults of the most recent run (for test.py)
TRACE = False


def kernel(x, state, b0, b1, a1):
    global LAST_RESULT
    try:
        import bass_utils
    except ModuleNotFoundError:
        from concourse import bass_utils

    x = np.asarray(x, dtype=np.float32)
    state = np.asarray(state, dtype=np.float32)
    b0f = float(np.asarray(b0).reshape(-1)[0])
    b1f = float(np.asarray(b1).reshape(-1)[0])
    a1f = float(np.asarray(a1).reshape(-1)[0])
    a1c = min(max(a1f, -1.0), 1.0)

    assert x.shape == (T, B), x.shape

    n_seg, n_warm = _segment_plan(a1c)
    g, gl = _build_matrices(a1c, b0f, b1f)
    nc = _get_program(n_seg, n_warm)

    seg_batches, n_blocks = _plan_batches(n_seg, n_warm)
    in_maps = []
    for core in range(NCORES):
        sl = x[:, core * NCB : (core + 1) * NCB]
        # chunk images [NCHUNK, 128, NCB]: row 0 = state slot (zero), 1+m = x
        ch = np.zeros((NCHUNK, 128, NCB), dtype=np.float32)
        ch[:, 1:, :] = sl[: NCHUNK * L1].reshape(NCHUNK, L1, NCB)
        xp = np.zeros(((n_blocks + 1) * 128, DMA_BATCH * NCB), dtype=np.float32)
        for batches in seg_batches:
            for blk, batch in batches:
                j0 = batch[0][0]
                bs = len(batch)
                img = ch[j0 : j0 + bs].transpose(1, 0, 2).reshape(128, bs * NCB)
                xp[blk * 128 : (blk + 1) * 128, 0 : bs * NCB] = img
        xp[n_blocks * 128 + 1, 0:NCB] = sl[T - 1]
        in_maps.append({"xp": xp, "g": g, "gl": gl})

    res = bass_utils.run_bass_kernel_spmd(
        nc, in_maps, core_ids=list(range(NCORES)), trace=TRACE
    )
    LAST_RESULT = res

    outs = []
    fss = []
    for r in res.results:
        op = r["outp"]
        main = op[: NCHUNK * 128].reshape(NCHUNK, 128, NCB)[:, 1:, :].reshape(
            NCHUNK * L1, NCB
        )
        last_row = op[NCHUNK * 128 + 1 : NCHUNK * 128 + 2]
        outs.append(np.concatenate([main, last_row], axis=0))
        fss.append(op[NCHUNK * 128])
    out = np.concatenate(outs, axis=1)
    fs = np.concatenate(fss, axis=0)

    if np.any(state):
        # host-side rank-1 correction for nonzero initial state
        pw = np.power(np.float64(a1c), np.arange(T, dtype=np.float64)).astype(np.float32)
        out = out + pw[:, None] * state[None, :]
        fs = fs + np.float32(float(a1c) ** T) * state

    return np.ascontiguousarray(out.astype(np.float32)), fs.astype(np.float32)
